# revision 1
# baseline (speedup 1.0000x reference)
"""Trainium2 Bass kernel for nn_ConditionalLayer (moe_routing).

out[i] = x[i] @ W[cond[i]].T + b.sum(0)       x:[8192,1024] W:[16,1024,1024]

Strategy (expert-parallel, host-routed):
  - Host groups rows by cond value (the "shard the condition axis" strategy):
    each of the 8 cores owns 2 of the 16 experts and receives only the rows
    routed to them, padded per expert slot to a multiple of 128.
  - Host pre-transposes x and W so both matmul operands have the contraction
    dim (d) on SBUF partitions -> no on-device transposes.
  - Device: per row-tile [128 rows] x f-block [512 cols]: 8 accumulating
    matmuls over d-chunks; bias b.sum(0) fused into the PSUM->SBUF eviction.
  - Host scatters routed rows back to their original positions.
"""

import os
import sys

import numpy as np

_TRN_REPO = "/opt/trn_rl_repo"
if os.path.isdir(_TRN_REPO) and _TRN_REPO not in sys.path:
    sys.path.insert(0, _TRN_REPO)

B, D, C = 8192, 1024, 16
NCORES = 8
SLOTS = C // NCORES  # experts per core
P = 128
FBLK = 512  # psum bank / fp32 moving-operand limit
DK = D // P  # contraction chunks
NF = D // FBLK  # f blocks

# 'float32' (exact, PE at 1/4 rate) or 'float32r' (tf32-like, full rate)
MM_DTYPE = "float32r"
TRACE = False
LAST_RESULT = None
LAST_NC = None

_nc_cache = {}


def _make_tile_context_cls():
    import concourse.mybir as mybir
    from concourse import tile
    from concourse.vector_clock import ScopedClock

    class TileContextFix(tile.TileContext):
        """This walrus build rejects >1 sync-wait per instruction.  Tile's
        scheduler freely assigns several.  Split the extras onto preceding
        NOPs on the same engine (same-engine program order makes this
        equivalent), and likewise chain the tail drain's waits."""

        _ws_counter = 0

        def _split_multi_waits(self):
            nc = self.nc
            for bb in nc.m.functions[0].blocks:
                insts = list(bb.instructions)
                if not any(
                    i.sync_info
                    and i.sync_info.on_wait
                    and len(i.sync_info.on_wait) > 1
                    for i in insts
                ):
                    continue
                new_seq = []
                for inst in insts:
                    si = inst.sync_info
                    waits = (
                        list(si.on_wait) if (si is not None and si.on_wait) else []
                    )
                    if len(waits) > 1:
                        for w in waits[:-1]:
                            TileContextFix._ws_counter += 1
                            nop = mybir.InstNoOp(
                                name=f"I-waitsplit-{TileContextFix._ws_counter}",
                                engine=inst.engine,
                            )
                            nop.sync_info = mybir.SyncInfo(
                                on_wait=[w], on_update=[]
                            )
                            new_seq.append(nop)
                        inst.sync_info = mybir.SyncInfo(
                            on_wait=[waits[-1]],
                            on_update=list(si.on_update) if si.on_update else [],
                        )
                    new_seq.append(inst)
                bb.instructions[:] = new_seq

        def _drain_and_barrier(self, tick_clock, wait_clock):
            self._split_multi_waits()
            drain_inst = self.nc.sync.drain()
            wait_clock.add_sem_waits(
                drain_inst.ins, ScopedClock({None: tick_clock.global_clock})
            )
            si = drain_inst.ins.sync_info
            waits = list(si.on_wait) if si is not None else []
            if len(waits) > 1:
                drain_inst.ins.sync_info = mybir.SyncInfo(
                    on_wait=waits[:1],
                    on_update=list(si.on_update) if si.on_update else [],
                )
                for w in waits[1:]:
                    extra = self.nc.sync.drain()
                    extra.ins.sync_info = mybir.SyncInfo(on_wait=[w], on_update=[])
            self.nc.all_engine_barrier()
            assert self.sems is not None
            popped = self.nc._tile_sem_poison_stack.pop()
            assert popped is self._sem_poison
            self.nc.clear_and_free_semaphores(list(self.sems.allocated().values()))
            self.nc.all_engine_barrier()

    return TileContextFix


def _tiles_of(M0, M1):
    """Row-tiles [(slot, col0, rows)]: full 128-row tiles + one ragged
    tile per slot."""
    tiles = []
    for s, (base, M) in enumerate(((0, M0), (M0, M1))):
        c = 0
        while c < M:
            r = min(P, M - c)
            tiles.append((s, base + c, r))
            c += r
    return tiles


def _build(M0, M1, mm_dtype):
    key = (M0, M1, mm_dtype)
    if key in _nc_cache:
        return _nc_cache[key]

    import concourse.bass as bass
    import concourse.mybir as mybir

    TileContextFix = _make_tile_context_cls()

    NTOT = M0 + M1
    # x columns padded so every 128-wide tile load stays in bounds
    NXPAD = M0 + P * (-(-M1 // P))
    nc = bass.Bass()
    mmdt = getattr(mybir.dt, mm_dtype)
    # x^T: [d, n] with routed rows as columns (slot0 block then slot1);
    # W pre-tiled on host as [slot][fb][dk] 128x512 contiguous blocks
    xT = nc.declare_dram_parameter("xT", [D, NXPAD], mmdt, isOutput=False)
    wt = nc.declare_dram_parameter(
        "wt", [SLOTS, NF, DK, P, FBLK], mmdt, isOutput=False
    )
    bias = nc.declare_dram_parameter("bias", [P, D], mybir.dt.float32, isOutput=False)
    out = nc.declare_dram_parameter("out", [NTOT, D], mybir.dt.float32, isOutput=True)

    tiles = _tiles_of(M0, M1)
    _SPLIT_LAST = globals().get("_SPLIT_LAST", 4)
    n_slot0 = sum(1 for s, _, _ in tiles if s == 0)
    WSPLIT = globals().get("_WSPLIT", 1)  # whole-W DMAs schedule best

    with TileContextFix(nc) as tc:
        with (
            tc.tile_pool(name="wpool", bufs=1) as wpool,
            tc.tile_pool(name="xpool", bufs=1) as xpool,
            tc.tile_pool(name="bpool", bufs=1) as bpool,
            tc.tile_pool(name="psum", bufs=6, space="PSUM") as pp,
            tc.tile_pool(name="opool", bufs=4) as op,
        ):
            # HWDGE descriptor generation is a serial ~625ns/DMA resource:
            # batch aggressively.  One DMA per x row-tile (all dk chunks,
            # always 128 cols -> full-width 512B descriptors), W in 1MB
            # half-blocks, one (ragged) store per row-tile.
            x_tiles = {}

            def load_x(t):
                _, c0, r = tiles[t]
                tl = xpool.tile([P, DK * P], mmdt, tag=f"x{t}")
                src = xT[:, c0 : c0 + P].rearrange("(dk p) m -> p dk m", p=P)
                nc.sync.dma_start(tl[:], src)
                x_tiles[t] = tl

            w_tiles = {}

            def load_w_half(s, fb, i):
                step = DK // WSPLIT
                tl = wpool.tile([P, step * FBLK], mmdt, tag=f"w{s}_{fb}_{i}")
                nc.sync.dma_start(
                    tl[:],
                    wt[s, fb, i * step : (i + 1) * step].rearrange(
                        "dk p f -> p dk f"
                    ),
                )
                w_tiles.setdefault((s, fb), [None] * WSPLIT)[i] = tl

            def load_w(s, fb):
                for i in range(WSPLIT):
                    load_w_half(s, fb, i)

            def w_slice(s, fb, dk):
                step = DK // WSPLIT
                tl = w_tiles[(s, fb)][dk // step]
                d = dk % step
                return tl[:, d * FBLK : (d + 1) * FBLK]

            bias_t = bpool.tile([P, D], mybir.dt.float32, tag="bias")

            # issue order = pipeline order: first psum group needs w(0,0)
            # and x(0); slot-1 weights land before the PE reaches the
            # slot-1 tiles, so the PE ramps early and stays fed.
            # bias first: the DVE evictions read bias_t, so it must be
            # resident before the first psum group retires or the psum
            # pool backs up and stalls the PE; later placements measure
            # strictly worse
            nc.sync.dma_start(bias_t[:], bias[:])
            load_w(0, 0)
            load_x(0)
            load_x(1)
            load_w(0, 1)
            for t in range(2, min(n_slot0 + 1, len(tiles))):
                load_x(t)
            load_w(1, 0)
            load_w(1, 1)
            for t in range(n_slot0 + 1, len(tiles)):
                load_x(t)

            for t, (s, c0, r) in enumerate(tiles):
                ot = op.tile([P, D], mybir.dt.float32, tag="o")
                for fb in range(NF):
                    ps = pp.tile([P, FBLK], mybir.dt.float32, tag="ps")
                    for dk in range(DK):
                        nc.tensor.matmul(
                            ps[:r, :],
                            x_tiles[t][:, dk * P : dk * P + r],
                            w_slice(s, fb, dk),
                            start=(dk == 0),
                            stop=(dk == DK - 1),
                        )
                    nc.vector.tensor_add(
                        ot[:r, fb * FBLK : (fb + 1) * FBLK],
                        ps[:r, :],
                        bias_t[:r, fb * FBLK : (fb + 1) * FBLK],
                    )
                # store issued from the otherwise-idle ACT engine so its
                # waits never head-of-line block the SP load stream; the
                # final two tiles store per-fb so their first halves
                # stream out before the last psum group retires
                if t >= len(tiles) - _SPLIT_LAST:
                    for fb in range(NF):
                        nc.scalar.dma_start(
                            out[c0 : c0 + r, fb * FBLK : (fb + 1) * FBLK],
                            ot[:r, fb * FBLK : (fb + 1) * FBLK],
                        )
                else:
                    nc.scalar.dma_start(out[c0 : c0 + r, :], ot[:r, :])

    _nc_cache[key] = nc
    return nc


def kernel(x, cond, W, b):
    from concourse.bass_utils import run_bass_kernel_spmd

    global LAST_RESULT, LAST_NC

    x = np.ascontiguousarray(np.asarray(x, dtype=np.float32))
    cond_i = np.asarray(cond).astype(np.int64)
    W = np.asarray(W, dtype=np.float32)
    b = np.asarray(b, dtype=np.float32)

    counts = np.bincount(cond_i, minlength=C)
    # Largest 8 experts -> slot 0, rest -> slot 1, so per-slot padding
    # (max count over that slot) is minimal.
    order = np.argsort(-counts, kind="stable")
    slot_experts = (order[:NCORES], order[NCORES:])
    M0 = max(1, int(counts[slot_experts[0]].max()))
    M1 = max(1, int(counts[slot_experts[1]].max()))
    NXPAD = M0 + P * (-(-M1 // P))

    nc = _build(M0, M1, MM_DTYPE)
    LAST_NC = nc

    bias_np = np.ascontiguousarray(
        np.broadcast_to(b.sum(axis=0).astype(np.float32), (P, D))
    )

    idx_by_e = [np.nonzero(cond_i == e)[0] for e in range(C)]
    in_maps = []
    placements = []
    for k in range(NCORES):
        xTk = np.zeros((D, NXPAD), np.float32)
        wTk = np.empty((SLOTS, D, D), np.float32)
        for s, col in enumerate((0, M0)):
            e = int(slot_experts[s][k])
            idx = idx_by_e[e]
            xTk[:, col : col + len(idx)] = x[idx].T
            wTk[s] = W[e].T
            placements.append((k, col, e))
        # [S, D, D] -> [S, NF, DK, 128, 512] contiguous blocks
        wtk = np.ascontiguousarray(
            wTk.reshape(SLOTS, DK, P, NF, FBLK).transpose(0, 3, 1, 2, 4)
        )
        in_maps.append({"xT": xTk, "wt": wtk, "bias": bias_np})

    res = run_bass_kernel_spmd(
        nc, in_maps, list(range(NCORES)), trace=TRACE
    )
    LAST_RESULT = res

    out_full = np.empty((B, D), np.float32)
    for k, col, e in placements:
        idx = idx_by_e[e]
        out_full[idx] = res.results[k]["out"][col : col + len(idx)]
    return out_full


if __name__ == "__main__":
    rng = np.random.default_rng(0)
    x = rng.standard_normal((B, D), dtype=np.float32)
    cond = rng.integers(0, C, size=B).astype(np.int64)
    W = (rng.standard_normal((C, D, D), dtype=np.float32) / np.sqrt(D)).astype(
        np.float32
    )
    b = (rng.standard_normal((C, D), dtype=np.float32) * 0.02).astype(np.float32)
    got = kernel(x, cond, W, b)
    want = np.empty((B, D), np.float32)
    for e in range(C):
        idx = np.nonzero(cond == e)[0]
        want[idx] = x[idx] @ W[e].T
    want += b.sum(0)
    denom = np.abs(want).max()
    print("max abs err:", np.abs(got - want).max(), "denom:", denom)
    print("rel err:", np.abs(got - want).max() / denom)



# revision 23
# speedup vs baseline: 1.4134x; 1.4134x over previous
"""Trainium2 Bass kernel for nn_ConditionalLayer (moe_routing).

out[i] = x[i] @ W[cond[i]].T + b.sum(0)       x:[8192,1024] W:[16,1024,1024]

Strategy (expert-parallel, host-routed, bf16):
  - Host groups rows by cond value: each of the 8 cores owns 2 of the 16
    experts (slot0 = one of the 8 largest, slot1 = one of the 8 smallest)
    and receives only the rows routed to them, padded to whole 128-row
    tiles with zeros.
  - Everything crossing HBM is bf16 (x, W, out) -> half the DMA traffic
    of fp32 at the same PE matmul rate.
  - Host pre-transposes x and W into DMA-friendly blocks: every DMA's
    innermost contiguous run is >= 512B (full-rate descriptors).
  - Device: 256-column sweeps over the tiles, f-major per slot, so the
    PE can start streaming after just 512KB of W; W DMAs are chunked so
    delivery tracks consumption.
  - The bias vector is loaded as a single 2KB row and broadcast across
    partitions by the PE (ones-column x bias-row matmul into PSUM, then
    evicted to SBUF) during the load phase -- no 512KB broadcast DMA.
  - PE p-state warmup: dummy matmuls on a memset tile ramp the tensor
    engine to full clock while the first DMAs land; the bias-broadcast
    matmuls are placed mid-warmup when their operand has landed.
  - PSUM->SBUF eviction fuses the bias add (DVE); stores ride the
    otherwise-idle ACT engine; the final group is narrow and stored
    from SP to shorten the tail chain.
  - Host scatters routed rows back to their original positions (fp32).
"""

import os
import sys

import numpy as np

_TRN_REPO = "/opt/trn_rl_repo"
if os.path.isdir(_TRN_REPO) and _TRN_REPO not in sys.path:
    sys.path.insert(0, _TRN_REPO)

B, D, C = 8192, 1024, 16
NCORES = 8
SLOTS = C // NCORES  # experts per core
P = 128
SW = 256  # sweep width (psum group columns)
NSW = D // SW  # sweeps per slot
FBLK = 512  # store block width
DK = D // P  # contraction chunks

N_WARM = 30  # PE p-state warmup matmuls (~107ns each at mid clock)
TRACE = False
LAST_RESULT = None
LAST_NC = None

_nc_cache = {}


def _make_tile_context_cls():
    import concourse.mybir as mybir
    from concourse import tile
    from concourse.vector_clock import ScopedClock

    class TileContextFix(tile.TileContext):
        """This walrus build rejects >1 sync-wait per instruction.  Tile's
        scheduler freely assigns several.  Split the extras onto preceding
        NOPs on the same engine (same-engine program order makes this
        equivalent), and likewise chain the tail drain's waits."""

        _ws_counter = 0

        def _split_multi_waits(self):
            nc = self.nc
            for bb in nc.m.functions[0].blocks:
                insts = list(bb.instructions)
                if not any(
                    i.sync_info
                    and i.sync_info.on_wait
                    and len(i.sync_info.on_wait) > 1
                    for i in insts
                ):
                    continue
                new_seq = []
                for inst in insts:
                    si = inst.sync_info
                    waits = (
                        list(si.on_wait) if (si is not None and si.on_wait) else []
                    )
                    if len(waits) > 1:
                        for w in waits[:-1]:
                            TileContextFix._ws_counter += 1
                            nop = mybir.InstNoOp(
                                name=f"I-waitsplit-{TileContextFix._ws_counter}",
                                engine=inst.engine,
                            )
                            nop.sync_info = mybir.SyncInfo(
                                on_wait=[w], on_update=[]
                            )
                            new_seq.append(nop)
                        inst.sync_info = mybir.SyncInfo(
                            on_wait=[waits[-1]],
                            on_update=list(si.on_update) if si.on_update else [],
                        )
                    new_seq.append(inst)
                bb.instructions[:] = new_seq

        def _drain_and_barrier(self, tick_clock, wait_clock):
            self._split_multi_waits()
            drain_inst = self.nc.sync.drain()
            wait_clock.add_sem_waits(
                drain_inst.ins, ScopedClock({None: tick_clock.global_clock})
            )
            si = drain_inst.ins.sync_info
            waits = list(si.on_wait) if si is not None else []
            if len(waits) > 1:
                drain_inst.ins.sync_info = mybir.SyncInfo(
                    on_wait=waits[:1],
                    on_update=list(si.on_update) if si.on_update else [],
                )
                for w in waits[1:]:
                    extra = self.nc.sync.drain()
                    extra.ins.sync_info = mybir.SyncInfo(on_wait=[w], on_update=[])
            self.nc.all_engine_barrier()
            assert self.sems is not None
            popped = self.nc._tile_sem_poison_stack.pop()
            assert popped is self._sem_poison
            self.nc.clear_and_free_semaphores(list(self.sems.allocated().values()))
            self.nc.all_engine_barrier()

    return TileContextFix


def _build(M0, M1):
    """Program for M0 slot-0 rows and M1 slot-1 rows per core (padded to
    128-row tiles; ragged tiles store only their real rows)."""
    key = (M0, M1, N_WARM)
    if key in _nc_cache:
        return _nc_cache[key]

    import concourse.bass as bass
    import concourse.mybir as mybir

    TileContextFix = _make_tile_context_cls()

    T0 = -(-M0 // P)
    T1 = -(-M1 // P)
    T = T0 + T1
    rows_of = {}
    for t in range(T0):
        rows_of[t] = min(P, M0 - t * P)
    for j in range(T1):
        rows_of[T0 + j] = min(P, M1 - j * P)

    nc = bass.Bass()
    bf = mybir.dt.bfloat16
    f32 = mybir.dt.float32
    # x pre-tiled on host: tile t, partition p (= contraction d % 128),
    # free [dk, m] -> value x[row m of tile t, dk*128+p]
    xt = nc.declare_dram_parameter("xt", [T, P, DK * P], bf, isOutput=False)
    # W pre-tiled on host as [slot][sweep][dk] 128x256 contiguous blocks
    wt = nc.declare_dram_parameter(
        "wt", [SLOTS, NSW, DK, P, SW], bf, isOutput=False
    )
    out = nc.declare_dram_parameter("out", [T * P, D], bf, isOutput=True)

    slot_tiles = (list(range(T0)), list(range(T0, T)))

    with TileContextFix(nc) as tc:
        with (
            tc.tile_pool(name="wpool", bufs=1) as wpool,
            tc.tile_pool(name="xpool", bufs=1) as xpool,
            tc.tile_pool(name="warm", bufs=1) as wmp,
            tc.tile_pool(name="psum", bufs=6, space="PSUM") as pp,
            tc.tile_pool(name="opool", bufs=7) as op,
        ):
            # --- loads (SP engine).  Transfers serialize on the DMA
            # engines, so order = need-order; the first W block is split
            # so the PE can start as soon as half of it has landed.
            x_tiles = {}

            def load_x(t):
                tl = xpool.tile([P, DK * P], bf, tag=f"x{t}")
                nc.sync.dma_start(tl[:], xt[t])
                x_tiles[t] = tl

            w_tiles = {}

            def load_w(s, j, nchunks=1):
                step = DK // nchunks
                for i in range(nchunks):
                    tl = wpool.tile([P, step * SW], bf, tag=f"w{s}_{j}_{i}")
                    nc.sync.dma_start(
                        tl[:],
                        wt[s, j, i * step : (i + 1) * step].rearrange(
                            "dk p f -> p dk f"
                        ),
                    )
                    w_tiles.setdefault((s, j), [None] * nchunks)[i] = tl

            def w_slice(s, j, dk):
                chunks = w_tiles[(s, j)]
                step = DK // len(chunks)
                return chunks[dk // step][:, (dk % step) * SW : (dk % step + 1) * SW]

            load_x(0)
            load_w(0, 0, 2)
            for t in range(1, T0):
                load_x(t)
            load_w(0, 1, 2)
            load_w(0, 2)
            load_w(0, 3)
            load_w(1, 0)
            for t in range(T0, T):
                load_x(t)
            load_w(1, 1)
            load_w(1, 2)
            load_w(1, 3)

            # --- PE p-state warmup on a memset tile: ramps the tensor
            # engine to full clock while the first DMAs land.  (The bias
            # add happens on the host after the gather, not on-device.)
            dum = wmp.tile([P, P], bf, tag="dum")
            nc.vector.memset(dum[:], 1.0)
            psd = pp.tile([P, SW], f32, tag="ps")
            for _ in range(N_WARM):
                nc.tensor.matmul(
                    psd[:, :P], dum[:], dum[:], start=True, stop=True
                )

            # --- compute: 256-column sweeps, f-major per slot.  Evictions
            # pair two sweeps into one 512-wide output tile; stores ride
            # the otherwise-idle ACT engine except the narrow tail store.
            ot_map = {}

            def group(s, t, j, c0, cw, store_to=None, last=False):
                r = rows_of[t]
                ps = pp.tile([P, SW], f32, tag="ps")
                for dk in range(DK):
                    nc.tensor.matmul(
                        ps[:, :cw],
                        x_tiles[t][:, dk * P : (dk + 1) * P],
                        w_slice(s, j, dk)[:, c0 : c0 + cw],
                        start=(dk == 0),
                        stop=(dk == DK - 1),
                    )
                if j % 2 == 0 and c0 == 0:
                    otl = op.tile([P, FBLK], bf, tag="o")
                    ot_map[t] = otl
                ot = ot_map[t]
                oc = (j % 2) * SW + c0
                fc = j * SW + c0  # global f column
                nc.vector.tensor_copy(ot[:r, oc : oc + cw], ps[:r, :cw])
                if store_to is not None:
                    # store [a, b) of the 512-wide output tile; the tail
                    # store uses SP (shorter DGE path, loads long done)
                    a, b_ = store_to
                    base = (j - j % 2) * SW
                    eng = nc.sync if last else nc.scalar
                    eng.dma_start(
                        out[t * P : t * P + r, base + a : base + b_],
                        ot[:r, a:b_],
                    )

            for s in range(SLOTS):
                for j in range(NSW):
                    for t in slot_tiles[s]:
                        if s == SLOTS - 1 and j == NSW - 1 and t == T - 1:
                            # final sweep group split: the wide part's
                            # eviction+store overlaps the narrow tail
                            # group's matmuls, shortening the tail chain
                            group(
                                s, t, j, 0, SW - P,
                                store_to=(0, FBLK - P), last=True,
                            )
                            group(
                                s, t, j, SW - P, P,
                                store_to=(FBLK - P, FBLK), last=True,
                            )
                        elif j % 2 == 1:
                            group(s, t, j, 0, SW, store_to=(0, FBLK))
                        else:
                            group(s, t, j, 0, SW)

    _nc_cache[key] = nc
    return nc


def _route(cond_i):
    """Expert->slot assignment and per-slot row counts from the routing."""
    counts = np.bincount(cond_i, minlength=C)
    order = np.argsort(-counts, kind="stable")
    slot_experts = (order[:NCORES], order[NCORES:])
    M0 = max(1, int(counts[slot_experts[0]].max()))
    M1 = max(1, int(counts[slot_experts[1]].max()))
    return slot_experts, M0, M1


def build_for_cond(cond):
    """Build (without running) the Bass module for the given routing."""
    cond_i = np.asarray(cond).astype(np.int64)
    _, M0, M1 = _route(cond_i)
    return _build(M0, M1)


def kernel(x, cond, W, b):
    import ml_dtypes

    from concourse.bass_utils import run_bass_kernel_spmd

    global LAST_RESULT, LAST_NC

    bf = ml_dtypes.bfloat16
    x = np.ascontiguousarray(np.asarray(x, dtype=np.float32))
    cond_i = np.asarray(cond).astype(np.int64)
    W = np.asarray(W, dtype=np.float32)
    b = np.asarray(b, dtype=np.float32)

    slot_experts, M0, M1 = _route(cond_i)
    T0 = -(-M0 // P)
    T1 = -(-M1 // P)
    T = T0 + T1

    nc = _build(M0, M1)
    LAST_NC = nc

    idx_by_e = [np.nonzero(cond_i == e)[0] for e in range(C)]
    in_maps = []
    placements = []
    for k in range(NCORES):
        xrows = np.zeros((T * P, D), np.float32)
        wtk = np.empty((SLOTS, NSW, DK, P, SW), bf)
        for s, base in enumerate((0, T0 * P)):
            e = int(slot_experts[s][k])
            idx = idx_by_e[e]
            xrows[base : base + len(idx)] = x[idx]
            # wt[s, j, dk, p, f] = W[e][j*256+f, dk*128+p]
            wtk[s] = (
                W[e]
                .T.reshape(DK, P, NSW, SW)
                .transpose(2, 0, 1, 3)
                .astype(bf)
            )
            placements.append((k, base, e))
        # xt[t, p, dk*128+m] = xrows[t*128+m, dk*128+p]
        xtk = np.ascontiguousarray(
            xrows.reshape(T, P, DK, P).transpose(0, 3, 2, 1).reshape(T, P, DK * P)
        ).astype(bf)
        in_maps.append({"xt": xtk, "wt": np.ascontiguousarray(wtk)})

    res = run_bass_kernel_spmd(nc, in_maps, list(range(NCORES)), trace=TRACE)
    LAST_RESULT = res

    out_full = np.empty((B, D), np.float32)
    for k, base, e in placements:
        idx = idx_by_e[e]
        out_full[idx] = res.results[k]["out"][base : base + len(idx)].astype(
            np.float32
        )
    out_full += b.sum(axis=0)
    return out_full


if __name__ == "__main__":
    rng = np.random.default_rng(0)
    x = rng.standard_normal((B, D), dtype=np.float32)
    cond = rng.integers(0, C, size=B).astype(np.int64)
    W = (rng.standard_normal((C, D, D), dtype=np.float32) / np.sqrt(D)).astype(
        np.float32
    )
    b = (rng.standard_normal((C, D), dtype=np.float32) * 0.02).astype(np.float32)
    got = kernel(x, cond, W, b)
    want = np.empty((B, D), np.float32)
    for e in range(C):
        idx = np.nonzero(cond == e)[0]
        want[idx] = x[idx] @ W[e].T
    want += b.sum(0)
    denom = np.abs(want).max()
    print("max abs err:", np.abs(got - want).max(), "denom:", denom)
    print("rel err:", np.abs(got - want).max() / denom)


# revision 33
# speedup vs baseline: 1.4448x; 1.0222x over previous
"""Trainium2 Bass kernel for nn_ConditionalLayer (moe_routing).

out[i] = x[i] @ W[cond[i]].T + b.sum(0)       x:[8192,1024] W:[16,1024,1024]

Strategy (expert-parallel, host-routed, bf16):
  - Host groups rows by cond value: each of the 8 cores owns 2 of the 16
    experts (slot0 = one of the 8 largest, slot1 = one of the 8 smallest)
    and receives only the rows routed to them, padded to whole 128-row
    tiles with zeros.
  - Everything crossing HBM is bf16 (x, W, out) -> half the DMA traffic
    of fp32 at the same PE matmul rate.
  - Host pre-transposes x and W into DMA-friendly blocks: every DMA's
    innermost contiguous run is >= 512B (full-rate descriptors).
  - Device: 256-column sweeps over the tiles, f-major per slot, so the
    PE can start streaming after just 512KB of W; W DMAs are chunked so
    delivery tracks consumption.
  - The bias vector is loaded as a single 2KB row and broadcast across
    partitions by the PE (ones-column x bias-row matmul into PSUM, then
    evicted to SBUF) during the load phase -- no 512KB broadcast DMA.
  - PE p-state warmup: dummy matmuls on a memset tile ramp the tensor
    engine to full clock while the first DMAs land; the bias-broadcast
    matmuls are placed mid-warmup when their operand has landed.
  - PSUM->SBUF eviction fuses the bias add (DVE); stores ride the
    otherwise-idle ACT engine; the final group is narrow and stored
    from SP to shorten the tail chain.
  - Host scatters routed rows back to their original positions (fp32).
"""

import os
import sys

import numpy as np

_TRN_REPO = "/opt/trn_rl_repo"
if os.path.isdir(_TRN_REPO) and _TRN_REPO not in sys.path:
    sys.path.insert(0, _TRN_REPO)

B, D, C = 8192, 1024, 16
NCORES = 8
SLOTS = C // NCORES  # experts per core
P = 128
SW = 256  # sweep width (psum group columns)
NSW = D // SW  # sweeps per slot
FBLK = 512  # store block width
DK = D // P  # contraction chunks

N_WARM = 30  # PE p-state warmup matmuls (~107ns each at mid clock)
TRACE = False
LAST_RESULT = None
LAST_NC = None

_nc_cache = {}


def _make_tile_context_cls():
    import concourse.mybir as mybir
    from concourse import tile
    from concourse.vector_clock import ScopedClock

    class TileContextFix(tile.TileContext):
        """This walrus build rejects >1 sync-wait per instruction.  Tile's
        scheduler freely assigns several.  Split the extras onto preceding
        NOPs on the same engine (same-engine program order makes this
        equivalent), and likewise chain the tail drain's waits."""

        _ws_counter = 0

        def _split_multi_waits(self):
            nc = self.nc
            for bb in nc.m.functions[0].blocks:
                insts = list(bb.instructions)
                if not any(
                    i.sync_info
                    and i.sync_info.on_wait
                    and len(i.sync_info.on_wait) > 1
                    for i in insts
                ):
                    continue
                new_seq = []
                for inst in insts:
                    si = inst.sync_info
                    waits = (
                        list(si.on_wait) if (si is not None and si.on_wait) else []
                    )
                    if len(waits) > 1:
                        for w in waits[:-1]:
                            TileContextFix._ws_counter += 1
                            nop = mybir.InstNoOp(
                                name=f"I-waitsplit-{TileContextFix._ws_counter}",
                                engine=inst.engine,
                            )
                            nop.sync_info = mybir.SyncInfo(
                                on_wait=[w], on_update=[]
                            )
                            new_seq.append(nop)
                        inst.sync_info = mybir.SyncInfo(
                            on_wait=[waits[-1]],
                            on_update=list(si.on_update) if si.on_update else [],
                        )
                    new_seq.append(inst)
                bb.instructions[:] = new_seq

        def _drain_and_barrier(self, tick_clock, wait_clock):
            self._split_multi_waits()
            drain_inst = self.nc.sync.drain()
            wait_clock.add_sem_waits(
                drain_inst.ins, ScopedClock({None: tick_clock.global_clock})
            )
            si = drain_inst.ins.sync_info
            waits = list(si.on_wait) if si is not None else []
            if len(waits) > 1:
                drain_inst.ins.sync_info = mybir.SyncInfo(
                    on_wait=waits[:1],
                    on_update=list(si.on_update) if si.on_update else [],
                )
                for w in waits[1:]:
                    extra = self.nc.sync.drain()
                    extra.ins.sync_info = mybir.SyncInfo(on_wait=[w], on_update=[])
            self.nc.all_engine_barrier()
            assert self.sems is not None
            popped = self.nc._tile_sem_poison_stack.pop()
            assert popped is self._sem_poison
            self.nc.clear_and_free_semaphores(list(self.sems.allocated().values()))

    return TileContextFix


def _build(M0, M1):
    """Program for M0 slot-0 rows and M1 slot-1 rows per core (padded to
    128-row tiles; ragged tiles store only their real rows)."""
    key = (M0, M1, N_WARM)
    if key in _nc_cache:
        return _nc_cache[key]

    import concourse.bass as bass
    import concourse.mybir as mybir

    TileContextFix = _make_tile_context_cls()

    T0 = -(-M0 // P)
    T1 = -(-M1 // P)
    T = T0 + T1
    rows_of = {}
    for t in range(T0):
        rows_of[t] = min(P, M0 - t * P)
    for j in range(T1):
        rows_of[T0 + j] = min(P, M1 - j * P)

    nc = bass.Bass()
    bf = mybir.dt.bfloat16
    f32 = mybir.dt.float32
    # x pre-tiled on host: tile t, partition p (= contraction d % 128),
    # free [dk, m] -> value x[row m of tile t, dk*128+p]
    xt = nc.declare_dram_parameter("xt", [T, P, DK * P], bf, isOutput=False)
    # W pre-tiled on host as [slot][sweep][dk] 128x256 contiguous blocks
    wt = nc.declare_dram_parameter(
        "wt", [SLOTS, NSW, DK, P, SW], bf, isOutput=False
    )
    out = nc.declare_dram_parameter("out", [T * P, D], bf, isOutput=True)
    identd = nc.declare_dram_parameter("ident", [P, P], bf, isOutput=False)

    # ragged tile first in each slot: its compact x tile is the smallest
    # load, which shortens the head anchor; the final tile stays full.
    slot_tiles = (
        [T0 - 1] + list(range(T0 - 1)),
        [T - 1] + list(range(T0, T - 1)),
    )
    # a sufficiently ragged slot-0 tile computes transposed (W stationary,
    # x moving: cost scales with its rows) and is re-transposed via the PE
    r0 = rows_of[T0 - 1]
    bt = T0 - 1 if r0 <= 104 else None  # orientation-B tile

    with TileContextFix(nc) as tc:
        with (
            tc.tile_pool(name="sb", bufs=1) as sb,
            tc.tile_pool(name="psum", bufs=6, space="PSUM") as pp,
            tc.tile_pool(name="opool", bufs=7) as op,
        ):
            wpool = xpool = wmp = sb
            # --- loads (SP engine).  Transfers serialize on the DMA
            # engines, so order = need-order; the first W block is split
            # so the PE can start as soon as half of it has landed.
            x_tiles = {}

            def load_x(t):
                r = rows_of[t]
                tl = xpool.tile([P, DK * r], bf, tag=f"x{t}")
                nc.sync.dma_start(tl[:], xt[t][:, : DK * r])
                x_tiles[t] = tl

            w_tiles = {}

            def load_w(s, j, nchunks=1):
                step = DK // nchunks
                for i in range(nchunks):
                    tl = wpool.tile([P, step * SW], bf, tag=f"w{s}_{j}_{i}")
                    nc.sync.dma_start(
                        tl[:],
                        wt[s, j, i * step : (i + 1) * step].rearrange(
                            "dk p f -> p dk f"
                        ),
                    )
                    w_tiles.setdefault((s, j), [None] * nchunks)[i] = tl

            def w_slice(s, j, dk):
                chunks = w_tiles[(s, j)]
                step = DK // len(chunks)
                return chunks[dk // step][:, (dk % step) * SW : (dk % step + 1) * SW]

            load_x(slot_tiles[0][0])
            load_w(0, 0, 2)
            for t in slot_tiles[0][1:]:
                load_x(t)
            load_w(0, 1, 2)
            load_w(0, 2)
            ident_t = None
            if bt is not None:
                ident_t = wmp.tile([P, P], bf, tag="ident")
                nc.sync.dma_start(ident_t[:], identd[:])
            load_w(0, 3)
            load_w(1, 0)
            for t in slot_tiles[1]:
                load_x(t)
            load_w(1, 1)
            load_w(1, 2)
            load_w(1, 3)

            # --- PE p-state warmup on a memset tile: ramps the tensor
            # engine to full clock while the first DMAs land.  (The bias
            # add happens on the host after the gather, not on-device.)
            dum = wmp.tile([P, P], bf, tag="dum")
            nc.vector.memset(dum[:], 1.0)
            psd = pp.tile([P, SW], f32, tag="ps")
            for _ in range(N_WARM):
                nc.tensor.matmul(
                    psd[:, :P], dum[:], dum[:], start=True, stop=True
                )

            # --- compute: 256-column sweeps, f-major per slot.  Evictions
            # pair two sweeps into one 512-wide output tile; stores ride
            # the otherwise-idle ACT engine except the narrow tail store.
            ot_map = {}

            def group(s, t, j, c0, cw, store_to=None, last=False):
                r = rows_of[t]
                ps = pp.tile([P, SW], f32, tag="ps")
                for dk in range(DK):
                    nc.tensor.matmul(
                        ps[:r, :cw],
                        x_tiles[t][:, dk * r : (dk + 1) * r],
                        w_slice(s, j, dk)[:, c0 : c0 + cw],
                        start=(dk == 0),
                        stop=(dk == DK - 1),
                    )
                if j % 2 == 0 and c0 == 0:
                    otl = op.tile([P, FBLK], bf, tag="o")
                    ot_map[t] = otl
                ot = ot_map[t]
                oc = (j % 2) * SW + c0
                fc = j * SW + c0  # global f column
                nc.vector.tensor_copy(ot[:r, oc : oc + cw], ps[:r, :cw])
                if store_to is not None:
                    # store [a, b) of the 512-wide output tile; the tail
                    # store uses SP (shorter DGE path, loads long done)
                    a, b_ = store_to
                    base = (j - j % 2) * SW
                    eng = nc.sync if last else nc.scalar
                    eng.dma_start(
                        out[t * P : t * P + r, base + a : base + b_],
                        ot[:r, a:b_],
                    )

            sbB = None
            if bt is not None:
                sbB = wmp.tile([P, 2 * NSW * r0], bf, tag="sbB")

            def group_b(j):
                """Ragged slot-0 tile, transposed orientation: W chunk is
                stationary, the tile's r0 rows are moving (cost ~r0 instead
                of the sweep width), landing [f, row] blocks in PSUM."""
                for fc in range(SW // P):
                    g = j * (SW // P) + fc
                    ps = pp.tile([P, SW], f32, tag="ps")
                    for dk in range(DK):
                        nc.tensor.matmul(
                            ps[:, :r0],
                            w_slice(0, j, dk)[:, fc * P : (fc + 1) * P],
                            x_tiles[bt][:, dk * r0 : (dk + 1) * r0],
                            start=(dk == 0),
                            stop=(dk == DK - 1),
                        )
                    nc.vector.tensor_copy(sbB[:, g * r0 : (g + 1) * r0], ps[:, :r0])

            def finish_b():
                """Re-transpose the 8 [128f, r0] blocks back to row-major
                via the PE and store the tile's rows."""
                otb = None
                for pair in range(D // SW):
                    psT = pp.tile([P, SW], bf, tag="ps")
                    for q in range(2):
                        g = pair * 2 + q
                        nc.tensor.transpose(
                            psT[:r0, q * P : (q + 1) * P],
                            sbB[:, g * r0 : (g + 1) * r0],
                            ident_t[:],
                        )
                    if pair % 2 == 0:
                        otb = op.tile([P, FBLK], bf, tag="o")
                    nc.vector.tensor_copy(
                        otb[:r0, (pair % 2) * SW : (pair % 2 + 1) * SW],
                        psT[:r0, :],
                    )
                    if pair % 2 == 1:
                        base = (pair - 1) * SW
                        nc.scalar.dma_start(
                            out[bt * P : bt * P + r0, base : base + FBLK],
                            otb[:r0, :],
                        )

            for s in range(SLOTS):
                for j in range(NSW):
                    for t in slot_tiles[s]:
                        if s == 0 and t == bt:
                            group_b(j)
                            continue
                        if (
                            s == SLOTS - 1
                            and j == NSW - 1
                            and t == slot_tiles[s][-1]
                        ):
                            # final sweep group split: the wide part's
                            # eviction+store overlaps the narrow tail
                            # group's matmuls, shortening the tail chain
                            group(
                                s, t, j, 0, SW - P,
                                store_to=(0, FBLK - P), last=True,
                            )
                            group(
                                s, t, j, SW - P, P,
                                store_to=(FBLK - P, FBLK), last=True,
                            )
                        elif j % 2 == 1:
                            group(s, t, j, 0, SW, store_to=(0, FBLK))
                        else:
                            group(s, t, j, 0, SW)
                        if (
                            s == 0
                            and j == NSW - 1
                            and bt is not None
                            and t == slot_tiles[0][1]
                        ):
                            # transpose pass one tile after the last B
                            # group, so its sbB eviction is long done
                            finish_b()

    _nc_cache[key] = nc
    return nc


def _route(cond_i):
    """Expert->slot assignment and per-slot row counts from the routing."""
    counts = np.bincount(cond_i, minlength=C)
    order = np.argsort(-counts, kind="stable")
    slot_experts = (order[:NCORES], order[NCORES:])
    M0 = max(1, int(counts[slot_experts[0]].max()))
    M1 = max(1, int(counts[slot_experts[1]].max()))
    return slot_experts, M0, M1


def build_for_cond(cond):
    """Build (without running) the Bass module for the given routing."""
    cond_i = np.asarray(cond).astype(np.int64)
    _, M0, M1 = _route(cond_i)
    return _build(M0, M1)


def kernel(x, cond, W, b):
    import ml_dtypes

    from concourse.bass_utils import run_bass_kernel_spmd

    global LAST_RESULT, LAST_NC

    bf = ml_dtypes.bfloat16
    x = np.ascontiguousarray(np.asarray(x, dtype=np.float32))
    cond_i = np.asarray(cond).astype(np.int64)
    W = np.asarray(W, dtype=np.float32)
    b = np.asarray(b, dtype=np.float32)

    slot_experts, M0, M1 = _route(cond_i)
    T0 = -(-M0 // P)
    T1 = -(-M1 // P)
    T = T0 + T1

    nc = _build(M0, M1)
    LAST_NC = nc

    idx_by_e = [np.nonzero(cond_i == e)[0] for e in range(C)]
    # program-wide rows held by each tile (ragged tails hold fewer)
    rows_prog = [min(P, M0 - t * P) for t in range(T0)] + [
        min(P, M1 - j * P) for j in range(T1)
    ]
    in_maps = []
    placements = []
    for k in range(NCORES):
        xtk = np.zeros((T, P, DK * P), bf)
        wtk = np.empty((SLOTS, NSW, DK, P, SW), bf)
        for s, base, tbase, Ts in ((0, 0, 0, T0), (1, T0 * P, T0, T1)):
            e = int(slot_experts[s][k])
            idx = idx_by_e[e]
            # wt[s, j, dk, p, f] = W[e][j*256+f, dk*128+p]
            wtk[s] = (
                W[e]
                .T.reshape(DK, P, NSW, SW)
                .transpose(2, 0, 1, 3)
                .astype(bf)
            )
            placements.append((k, base, e))
            for j in range(Ts):
                t = tbase + j
                rp = rows_prog[t]
                rows = idx[j * P : j * P + rp]
                if not len(rows):
                    continue
                # compact layout: xt[t, p, dk*rp + m] = x[rows[m], dk*128+p]
                blk = np.zeros((P, DK, rp), np.float32)
                blk[:, :, : len(rows)] = (
                    x[rows].reshape(len(rows), DK, P).transpose(2, 1, 0)
                )
                xtk[t, :, : DK * rp] = blk.reshape(P, DK * rp).astype(bf)
        in_maps.append(
            {
                "xt": xtk,
                "wt": np.ascontiguousarray(wtk),
                "ident": np.eye(P, dtype=bf),
            }
        )

    res = run_bass_kernel_spmd(nc, in_maps, list(range(NCORES)), trace=TRACE)
    LAST_RESULT = res

    out_full = np.empty((B, D), np.float32)
    for k, base, e in placements:
        idx = idx_by_e[e]
        out_full[idx] = res.results[k]["out"][base : base + len(idx)].astype(
            np.float32
        )
    out_full += b.sum(axis=0)
    return out_full


if __name__ == "__main__":
    rng = np.random.default_rng(0)
    x = rng.standard_normal((B, D), dtype=np.float32)
    cond = rng.integers(0, C, size=B).astype(np.int64)
    W = (rng.standard_normal((C, D, D), dtype=np.float32) / np.sqrt(D)).astype(
        np.float32
    )
    b = (rng.standard_normal((C, D), dtype=np.float32) * 0.02).astype(np.float32)
    got = kernel(x, cond, W, b)
    want = np.empty((B, D), np.float32)
    for e in range(C):
        idx = np.nonzero(cond == e)[0]
        want[idx] = x[idx] @ W[e].T
    want += b.sum(0)
    denom = np.abs(want).max()
    print("max abs err:", np.abs(got - want).max(), "denom:", denom)
    print("rel err:", np.abs(got - want).max() / denom)


# revision 40
# speedup vs baseline: 1.4559x; 1.0077x over previous
"""Trainium2 Bass kernel for nn_ConditionalLayer (moe_routing).

out[i] = x[i] @ W[cond[i]].T + b.sum(0)       x:[8192,1024] W:[16,1024,1024]

Strategy (expert-parallel, host-routed, bf16):
  - Host groups rows by cond value: each of the 8 cores owns 2 of the 16
    experts (slot0 = one of the 8 largest, slot1 = one of the 8 smallest)
    and receives only the rows routed to them, padded to whole 128-row
    tiles with zeros.
  - Everything crossing HBM is bf16 (x, W, out) -> half the DMA traffic
    of fp32 at the same PE matmul rate.
  - Host pre-transposes x and W into DMA-friendly blocks: every DMA's
    innermost contiguous run is >= 512B (full-rate descriptors).
  - Device: 256-column sweeps over the tiles, f-major per slot, so the
    PE can start streaming after just 512KB of W; W DMAs are chunked so
    delivery tracks consumption.
  - Ragged tail tiles load their x compactly (r rows, not 128) and run
    first in each slot's sweeps, shortening the head anchor; a very
    ragged slot-0 tile computes in transposed orientation (W stationary,
    x moving -- PE cost scales with its rows) and is re-transposed via
    the PE with an identity operand.
  - PE p-state warmup: dummy matmuls on a memset tile ramp the tensor
    engine to full clock while the first DMAs land.
  - PSUM->SBUF evictions are plain copies (DVE); the bias add happens on
    the host after the gather.  Stores ride the otherwise-idle ACT
    engine; the final tile stores each piece eagerly so the tail chain
    holds only one narrow 128-column store from SP.
  - Host scatters routed rows back to their original positions and adds
    b.sum(0) there (fp32).
"""

import os
import sys

import numpy as np

_TRN_REPO = "/opt/trn_rl_repo"
if os.path.isdir(_TRN_REPO) and _TRN_REPO not in sys.path:
    sys.path.insert(0, _TRN_REPO)

B, D, C = 8192, 1024, 16
NCORES = 8
SLOTS = C // NCORES  # experts per core
P = 128
SW = 256  # sweep width (psum group columns)
NSW = D // SW  # sweeps per slot
FBLK = 512  # store block width
DK = D // P  # contraction chunks

N_WARM = 30  # PE p-state warmup matmuls (~107ns each at mid clock)
TRACE = False
LAST_RESULT = None
LAST_NC = None

_nc_cache = {}


def _make_tile_context_cls():
    import concourse.mybir as mybir
    from concourse import tile
    from concourse.vector_clock import ScopedClock

    class TileContextFix(tile.TileContext):
        """This walrus build rejects >1 sync-wait per instruction.  Tile's
        scheduler freely assigns several.  Split the extras onto preceding
        NOPs on the same engine (same-engine program order makes this
        equivalent), and likewise chain the tail drain's waits."""

        _ws_counter = 0

        def _split_multi_waits(self):
            nc = self.nc
            for bb in nc.m.functions[0].blocks:
                insts = list(bb.instructions)
                if not any(
                    i.sync_info
                    and i.sync_info.on_wait
                    and len(i.sync_info.on_wait) > 1
                    for i in insts
                ):
                    continue
                new_seq = []
                for inst in insts:
                    si = inst.sync_info
                    waits = (
                        list(si.on_wait) if (si is not None and si.on_wait) else []
                    )
                    if len(waits) > 1:
                        for w in waits[:-1]:
                            TileContextFix._ws_counter += 1
                            nop = mybir.InstNoOp(
                                name=f"I-waitsplit-{TileContextFix._ws_counter}",
                                engine=inst.engine,
                            )
                            nop.sync_info = mybir.SyncInfo(
                                on_wait=[w], on_update=[]
                            )
                            new_seq.append(nop)
                        inst.sync_info = mybir.SyncInfo(
                            on_wait=[waits[-1]],
                            on_update=list(si.on_update) if si.on_update else [],
                        )
                    new_seq.append(inst)
                bb.instructions[:] = new_seq

        def _drain_and_barrier(self, tick_clock, wait_clock):
            self._split_multi_waits()
            drain_inst = self.nc.sync.drain()
            wait_clock.add_sem_waits(
                drain_inst.ins, ScopedClock({None: tick_clock.global_clock})
            )
            si = drain_inst.ins.sync_info
            waits = list(si.on_wait) if si is not None else []
            if len(waits) > 1:
                drain_inst.ins.sync_info = mybir.SyncInfo(
                    on_wait=waits[:1],
                    on_update=list(si.on_update) if si.on_update else [],
                )
                for w in waits[1:]:
                    extra = self.nc.sync.drain()
                    extra.ins.sync_info = mybir.SyncInfo(on_wait=[w], on_update=[])
            self.nc.all_engine_barrier()
            assert self.sems is not None
            popped = self.nc._tile_sem_poison_stack.pop()
            assert popped is self._sem_poison
            self.nc.clear_and_free_semaphores(list(self.sems.allocated().values()))

    return TileContextFix


def _build(M0, M1):
    """Program for M0 slot-0 rows and M1 slot-1 rows per core (padded to
    128-row tiles; ragged tiles store only their real rows)."""
    key = (M0, M1, N_WARM)
    if key in _nc_cache:
        return _nc_cache[key]

    import concourse.bass as bass
    import concourse.mybir as mybir

    TileContextFix = _make_tile_context_cls()

    T0 = -(-M0 // P)
    T1 = -(-M1 // P)
    T = T0 + T1
    rows_of = {}
    for t in range(T0):
        rows_of[t] = min(P, M0 - t * P)
    for j in range(T1):
        rows_of[T0 + j] = min(P, M1 - j * P)

    nc = bass.Bass()
    bf = mybir.dt.bfloat16
    f32 = mybir.dt.float32
    # x pre-tiled on host: tile t, partition p (= contraction d % 128),
    # free [dk, m] -> value x[row m of tile t, dk*128+p]
    xt = nc.declare_dram_parameter("xt", [T, P, DK * P], bf, isOutput=False)
    # W pre-tiled on host as [slot][sweep][dk] 128x256 contiguous blocks
    wt = nc.declare_dram_parameter(
        "wt", [SLOTS, NSW, DK, P, SW], bf, isOutput=False
    )
    out = nc.declare_dram_parameter("out", [T * P, D], bf, isOutput=True)
    identd = nc.declare_dram_parameter("ident", [P, P], bf, isOutput=False)

    # ragged tile first in each slot: its compact x tile is the smallest
    # load, which shortens the head anchor; the final tile stays full.
    slot_tiles = (
        [T0 - 1] + list(range(T0 - 1)),
        [T - 1] + list(range(T0, T - 1)),
    )
    # a sufficiently ragged slot-0 tile computes transposed (W stationary,
    # x moving: cost scales with its rows) and is re-transposed via the PE
    r0 = rows_of[T0 - 1]
    bt = T0 - 1 if r0 <= 104 else None  # orientation-B tile

    with TileContextFix(nc) as tc:
        with (
            tc.tile_pool(name="sb", bufs=1) as sb,
            tc.tile_pool(name="psum", bufs=6, space="PSUM") as pp,
            tc.tile_pool(name="opool", bufs=7) as op,
        ):
            wpool = xpool = wmp = sb
            # --- loads (SP engine).  Transfers serialize on the DMA
            # engines, so order = need-order; the first W block is split
            # so the PE can start as soon as half of it has landed.
            x_tiles = {}

            def load_x(t):
                r = rows_of[t]
                tl = xpool.tile([P, DK * r], bf, tag=f"x{t}")
                nc.sync.dma_start(tl[:], xt[t][:, : DK * r])
                x_tiles[t] = tl

            w_tiles = {}

            def load_w(s, j, nchunks=1):
                step = DK // nchunks
                for i in range(nchunks):
                    tl = wpool.tile([P, step * SW], bf, tag=f"w{s}_{j}_{i}")
                    nc.sync.dma_start(
                        tl[:],
                        wt[s, j, i * step : (i + 1) * step].rearrange(
                            "dk p f -> p dk f"
                        ),
                    )
                    w_tiles.setdefault((s, j), [None] * nchunks)[i] = tl

            def w_slice(s, j, dk):
                chunks = w_tiles[(s, j)]
                step = DK // len(chunks)
                return chunks[dk // step][:, (dk % step) * SW : (dk % step + 1) * SW]

            load_x(slot_tiles[0][0])
            load_w(0, 0, 2)
            for t in slot_tiles[0][1:]:
                load_x(t)
            load_w(0, 1, 2)
            load_w(0, 2)
            ident_t = None
            if bt is not None:
                ident_t = wmp.tile([P, P], bf, tag="ident")
                nc.sync.dma_start(ident_t[:], identd[:])
            load_w(0, 3)
            load_w(1, 0)
            for t in slot_tiles[1]:
                load_x(t)
            load_w(1, 1)
            load_w(1, 2)
            load_w(1, 3)

            # --- PE p-state warmup on a memset tile: ramps the tensor
            # engine to full clock while the first DMAs land.  (The bias
            # add happens on the host after the gather, not on-device.)
            dum = wmp.tile([P, P], bf, tag="dum")
            nc.vector.memset(dum[:], 1.0)
            psd = pp.tile([P, SW], f32, tag="ps")
            for _ in range(N_WARM):
                nc.tensor.matmul(
                    psd[:, :P], dum[:], dum[:], start=True, stop=True
                )

            # --- compute: 256-column sweeps, f-major per slot.  Evictions
            # pair two sweeps into one 512-wide output tile; stores ride
            # the otherwise-idle ACT engine except the narrow tail store.
            ot_map = {}

            def group(s, t, j, c0, cw, store_to=None, last=None):
                r = rows_of[t]
                ps = pp.tile([P, SW], f32, tag="ps")
                for dk in range(DK):
                    nc.tensor.matmul(
                        ps[:r, :cw],
                        x_tiles[t][:, dk * r : (dk + 1) * r],
                        w_slice(s, j, dk)[:, c0 : c0 + cw],
                        start=(dk == 0),
                        stop=(dk == DK - 1),
                    )
                if j % 2 == 0 and c0 == 0:
                    otl = op.tile([P, FBLK], bf, tag="o")
                    ot_map[t] = otl
                ot = ot_map[t]
                oc = (j % 2) * SW + c0
                fc = j * SW + c0  # global f column
                nc.vector.tensor_copy(ot[:r, oc : oc + cw], ps[:r, :cw])
                if store_to is not None:
                    # store [a, b) of the 512-wide output tile; the two
                    # tail stores split across SP and DVE so their SEQ+
                    # HWDGE paths don't serialize (loads long done)
                    a, b_ = store_to
                    base = (j - j % 2) * SW
                    eng = {None: nc.scalar, "sp": nc.sync, "act": nc.scalar}[last]
                    eng.dma_start(
                        out[t * P : t * P + r, base + a : base + b_],
                        ot[:r, a:b_],
                    )

            sbB = None
            if bt is not None:
                sbB = wmp.tile([P, 2 * NSW * r0], bf, tag="sbB")

            def group_b(j):
                """Ragged slot-0 tile, transposed orientation: W chunk is
                stationary, the tile's r0 rows are moving (cost ~r0 instead
                of the sweep width), landing [f, row] blocks in PSUM."""
                for fc in range(SW // P):
                    g = j * (SW // P) + fc
                    ps = pp.tile([P, SW], f32, tag="ps")
                    for dk in range(DK):
                        nc.tensor.matmul(
                            ps[:, :r0],
                            w_slice(0, j, dk)[:, fc * P : (fc + 1) * P],
                            x_tiles[bt][:, dk * r0 : (dk + 1) * r0],
                            start=(dk == 0),
                            stop=(dk == DK - 1),
                        )
                    nc.vector.tensor_copy(sbB[:, g * r0 : (g + 1) * r0], ps[:, :r0])

            def finish_b():
                """Re-transpose the 8 [128f, r0] blocks back to row-major
                via the PE and store the tile's rows."""
                otb = None
                for pair in range(D // SW):
                    psT = pp.tile([P, SW], bf, tag="ps")
                    for q in range(2):
                        g = pair * 2 + q
                        nc.tensor.transpose(
                            psT[:r0, q * P : (q + 1) * P],
                            sbB[:, g * r0 : (g + 1) * r0],
                            ident_t[:],
                        )
                    if pair % 2 == 0:
                        otb = op.tile([P, FBLK], bf, tag="o")
                    nc.vector.tensor_copy(
                        otb[:r0, (pair % 2) * SW : (pair % 2 + 1) * SW],
                        psT[:r0, :],
                    )
                    if pair % 2 == 1:
                        base = (pair - 1) * SW
                        nc.scalar.dma_start(
                            out[bt * P : bt * P + r0, base : base + FBLK],
                            otb[:r0, :],
                        )

            for s in range(SLOTS):
                for j in range(NSW):
                    for t in slot_tiles[s]:
                        if s == 0 and t == bt:
                            group_b(j)
                            if j == NSW - 1 and len(slot_tiles[0]) == 1:
                                finish_b()
                            continue
                        if (
                            s == SLOTS - 1
                            and j == NSW - 2
                            and t == slot_tiles[s][-1]
                        ):
                            # final tile: store each piece as soon as its
                            # eviction lands so the tail chain holds only
                            # the last narrow 128-column store
                            group(s, t, j, 0, SW, store_to=(0, SW))
                        elif (
                            s == SLOTS - 1
                            and j == NSW - 1
                            and t == slot_tiles[s][-1]
                        ):
                            group(s, t, j, 0, SW - P, store_to=(SW, FBLK - P), last="sp")
                            group(
                                s, t, j, SW - P, P,
                                store_to=(FBLK - P, FBLK), last="sp",
                            )
                        elif j % 2 == 1:
                            group(s, t, j, 0, SW, store_to=(0, FBLK))
                        else:
                            group(s, t, j, 0, SW)
                        if (
                            s == 0
                            and j == NSW - 1
                            and bt is not None
                            and t == slot_tiles[0][1]
                        ):
                            # transpose pass one tile after the last B
                            # group, so its sbB eviction is long done
                            finish_b()

    _nc_cache[key] = nc
    return nc


def _route(cond_i):
    """Expert->slot assignment and per-slot row counts from the routing."""
    counts = np.bincount(cond_i, minlength=C)
    order = np.argsort(-counts, kind="stable")
    slot_experts = (order[:NCORES], order[NCORES:])
    M0 = max(1, int(counts[slot_experts[0]].max()))
    M1 = max(1, int(counts[slot_experts[1]].max()))
    return slot_experts, M0, M1


def build_for_cond(cond):
    """Build (without running) the Bass module for the given routing."""
    cond_i = np.asarray(cond).astype(np.int64)
    _, M0, M1 = _route(cond_i)
    return _build(M0, M1)


def kernel(x, cond, W, b):
    import ml_dtypes

    from concourse.bass_utils import run_bass_kernel_spmd

    global LAST_RESULT, LAST_NC

    bf = ml_dtypes.bfloat16
    x = np.ascontiguousarray(np.asarray(x, dtype=np.float32))
    cond_i = np.asarray(cond).astype(np.int64)
    W = np.asarray(W, dtype=np.float32)
    b = np.asarray(b, dtype=np.float32)

    slot_experts, M0, M1 = _route(cond_i)
    T0 = -(-M0 // P)
    T1 = -(-M1 // P)
    T = T0 + T1

    nc = _build(M0, M1)
    LAST_NC = nc

    idx_by_e = [np.nonzero(cond_i == e)[0] for e in range(C)]
    # program-wide rows held by each tile (ragged tails hold fewer)
    rows_prog = [min(P, M0 - t * P) for t in range(T0)] + [
        min(P, M1 - j * P) for j in range(T1)
    ]
    in_maps = []
    placements = []
    for k in range(NCORES):
        xtk = np.zeros((T, P, DK * P), bf)
        wtk = np.empty((SLOTS, NSW, DK, P, SW), bf)
        for s, base, tbase, Ts in ((0, 0, 0, T0), (1, T0 * P, T0, T1)):
            e = int(slot_experts[s][k])
            idx = idx_by_e[e]
            # wt[s, j, dk, p, f] = W[e][j*256+f, dk*128+p]
            wtk[s] = (
                W[e]
                .T.reshape(DK, P, NSW, SW)
                .transpose(2, 0, 1, 3)
                .astype(bf)
            )
            placements.append((k, base, e))
            for j in range(Ts):
                t = tbase + j
                rp = rows_prog[t]
                rows = idx[j * P : j * P + rp]
                if not len(rows):
                    continue
                # compact layout: xt[t, p, dk*rp + m] = x[rows[m], dk*128+p]
                blk = np.zeros((P, DK, rp), np.float32)
                blk[:, :, : len(rows)] = (
                    x[rows].reshape(len(rows), DK, P).transpose(2, 1, 0)
                )
                xtk[t, :, : DK * rp] = blk.reshape(P, DK * rp).astype(bf)
        in_maps.append(
            {
                "xt": xtk,
                "wt": np.ascontiguousarray(wtk),
                "ident": np.eye(P, dtype=bf),
            }
        )

    res = run_bass_kernel_spmd(nc, in_maps, list(range(NCORES)), trace=TRACE)
    LAST_RESULT = res

    out_full = np.empty((B, D), np.float32)
    for k, base, e in placements:
        idx = idx_by_e[e]
        out_full[idx] = res.results[k]["out"][base : base + len(idx)].astype(
            np.float32
        )
    out_full += b.sum(axis=0)
    return out_full


if __name__ == "__main__":
    rng = np.random.default_rng(0)
    x = rng.standard_normal((B, D), dtype=np.float32)
    cond = rng.integers(0, C, size=B).astype(np.int64)
    W = (rng.standard_normal((C, D, D), dtype=np.float32) / np.sqrt(D)).astype(
        np.float32
    )
    b = (rng.standard_normal((C, D), dtype=np.float32) * 0.02).astype(np.float32)
    got = kernel(x, cond, W, b)
    want = np.empty((B, D), np.float32)
    for e in range(C):
        idx = np.nonzero(cond == e)[0]
        want[idx] = x[idx] @ W[e].T
    want += b.sum(0)
    denom = np.abs(want).max()
    print("max abs err:", np.abs(got - want).max(), "denom:", denom)
    print("rel err:", np.abs(got - want).max() / denom)


# revision 43
# speedup vs baseline: 1.4607x; 1.0033x over previous
"""Trainium2 Bass kernel for nn_ConditionalLayer (moe_routing).

out[i] = x[i] @ W[cond[i]].T + b.sum(0)       x:[8192,1024] W:[16,1024,1024]

Strategy (expert-parallel, host-routed, bf16):
  - Host groups rows by cond value: each of the 8 cores owns 2 of the 16
    experts (slot0 = one of the 8 largest, slot1 = one of the 8 smallest)
    and receives only the rows routed to them, padded to whole 128-row
    tiles with zeros.
  - Everything crossing HBM is bf16 (x, W, out) -> half the DMA traffic
    of fp32 at the same PE matmul rate.
  - Host pre-transposes x and W into DMA-friendly blocks: every DMA's
    innermost contiguous run is >= 512B (full-rate descriptors).
  - Device: 256-column sweeps over the tiles, f-major per slot, so the
    PE can start streaming after just 512KB of W; W DMAs are chunked so
    delivery tracks consumption.
  - Ragged tail tiles load their x compactly (r rows, not 128) and run
    first in each slot's sweeps, shortening the head anchor; a very
    ragged slot-0 tile computes in transposed orientation (W stationary,
    x moving -- PE cost scales with its rows) and is re-transposed via
    the PE with an identity operand.
  - PE p-state warmup: dummy matmuls on a memset tile ramp the tensor
    engine to full clock while the first DMAs land.
  - PSUM->SBUF evictions are plain copies (DVE); the bias add happens on
    the host after the gather.  Stores ride the otherwise-idle ACT
    engine; the final tile stores each piece eagerly so the tail chain
    holds only one narrow 128-column store from SP.
  - Host scatters routed rows back to their original positions and adds
    b.sum(0) there (fp32).
"""

import os
import sys

import numpy as np

_TRN_REPO = "/opt/trn_rl_repo"
if os.path.isdir(_TRN_REPO) and _TRN_REPO not in sys.path:
    sys.path.insert(0, _TRN_REPO)

B, D, C = 8192, 1024, 16
NCORES = 8
SLOTS = C // NCORES  # experts per core
P = 128
SW = 256  # sweep width (psum group columns)
NSW = D // SW  # sweeps per slot
FBLK = 512  # store block width
DK = D // P  # contraction chunks

N_WARM = 30  # PE p-state warmup matmuls (~107ns each at mid clock)
TRACE = False
LAST_RESULT = None
LAST_NC = None

_nc_cache = {}


def _make_tile_context_cls():
    import concourse.mybir as mybir
    from concourse import tile
    from concourse.vector_clock import ScopedClock

    class TileContextFix(tile.TileContext):
        """This walrus build rejects >1 sync-wait per instruction.  Tile's
        scheduler freely assigns several.  Split the extras onto preceding
        NOPs on the same engine (same-engine program order makes this
        equivalent), and likewise chain the tail drain's waits."""

        _ws_counter = 0

        def _split_multi_waits(self):
            nc = self.nc
            for bb in nc.m.functions[0].blocks:
                insts = list(bb.instructions)
                if not any(
                    i.sync_info
                    and i.sync_info.on_wait
                    and len(i.sync_info.on_wait) > 1
                    for i in insts
                ):
                    continue
                new_seq = []
                for inst in insts:
                    si = inst.sync_info
                    waits = (
                        list(si.on_wait) if (si is not None and si.on_wait) else []
                    )
                    if len(waits) > 1:
                        for w in waits[:-1]:
                            TileContextFix._ws_counter += 1
                            nop = mybir.InstNoOp(
                                name=f"I-waitsplit-{TileContextFix._ws_counter}",
                                engine=inst.engine,
                            )
                            nop.sync_info = mybir.SyncInfo(
                                on_wait=[w], on_update=[]
                            )
                            new_seq.append(nop)
                        inst.sync_info = mybir.SyncInfo(
                            on_wait=[waits[-1]],
                            on_update=list(si.on_update) if si.on_update else [],
                        )
                    new_seq.append(inst)
                bb.instructions[:] = new_seq

        def _drain_and_barrier(self, tick_clock, wait_clock):
            self._split_multi_waits()
            drain_inst = self.nc.sync.drain()
            wait_clock.add_sem_waits(
                drain_inst.ins, ScopedClock({None: tick_clock.global_clock})
            )
            si = drain_inst.ins.sync_info
            waits = list(si.on_wait) if si is not None else []
            if len(waits) > 1:
                drain_inst.ins.sync_info = mybir.SyncInfo(
                    on_wait=waits[:1],
                    on_update=list(si.on_update) if si.on_update else [],
                )
                for w in waits[1:]:
                    extra = self.nc.sync.drain()
                    extra.ins.sync_info = mybir.SyncInfo(on_wait=[w], on_update=[])
            self.nc.all_engine_barrier()
            assert self.sems is not None
            popped = self.nc._tile_sem_poison_stack.pop()
            assert popped is self._sem_poison
            self.nc.clear_and_free_semaphores(list(self.sems.allocated().values()))

    return TileContextFix


def _build(M0, M1):
    """Program for M0 slot-0 rows and M1 slot-1 rows per core (padded to
    128-row tiles; ragged tiles store only their real rows)."""
    key = (M0, M1, N_WARM)
    if key in _nc_cache:
        return _nc_cache[key]

    import concourse.bass as bass
    import concourse.mybir as mybir

    TileContextFix = _make_tile_context_cls()

    T0 = -(-M0 // P)
    T1 = -(-M1 // P)
    T = T0 + T1
    rows_of = {}
    for t in range(T0):
        rows_of[t] = min(P, M0 - t * P)
    for j in range(T1):
        rows_of[T0 + j] = min(P, M1 - j * P)

    nc = bass.Bass()
    bf = mybir.dt.bfloat16
    f32 = mybir.dt.float32
    # x pre-tiled on host: tile t, partition p (= contraction d % 128),
    # free [dk, m] -> value x[row m of tile t, dk*128+p]
    xt = nc.declare_dram_parameter("xt", [T, P, DK * P], bf, isOutput=False)
    # W pre-tiled on host as [slot][sweep][dk] 128x256 contiguous blocks
    wt = nc.declare_dram_parameter(
        "wt", [SLOTS, NSW, DK, P, SW], bf, isOutput=False
    )
    out = nc.declare_dram_parameter("out", [T * P, D], bf, isOutput=True)
    identd = nc.declare_dram_parameter("ident", [P, P], bf, isOutput=False)

    # ragged tile first in each slot: its compact x tile is the smallest
    # load, which shortens the head anchor; the final tile stays full.
    slot_tiles = (
        [T0 - 1] + list(range(T0 - 1)),
        [T - 1] + list(range(T0, T - 1)),
    )
    # a sufficiently ragged slot-0 tile computes transposed (W stationary,
    # x moving: cost scales with its rows) and is re-transposed via the PE
    r0 = rows_of[T0 - 1]
    bt = T0 - 1 if r0 <= 104 else None  # orientation-B tile

    with TileContextFix(nc) as tc:
        with (
            tc.tile_pool(name="sb", bufs=1) as sb,
            tc.tile_pool(name="psum", bufs=6, space="PSUM") as pp,
            tc.tile_pool(name="opool", bufs=7) as op,
        ):
            wpool = xpool = wmp = sb
            # --- loads (SP engine).  Transfers serialize on the DMA
            # engines, so order = need-order; the first W block is split
            # so the PE can start as soon as half of it has landed.
            x_tiles = {}

            def load_x(t):
                r = rows_of[t]
                tl = xpool.tile([P, DK * r], bf, tag=f"x{t}")
                nc.sync.dma_start(tl[:], xt[t][:, : DK * r])
                x_tiles[t] = tl

            w_tiles = {}

            def load_w(s, j, nchunks=1, upto=None, fromi=0):
                step = DK // nchunks
                for i in range(fromi, nchunks if upto is None else upto):
                    tl = wpool.tile([P, step * SW], bf, tag=f"w{s}_{j}_{i}")
                    nc.sync.dma_start(
                        tl[:],
                        wt[s, j, i * step : (i + 1) * step].rearrange(
                            "dk p f -> p dk f"
                        ),
                    )
                    w_tiles.setdefault((s, j), [None] * nchunks)[i] = tl

            def w_slice(s, j, dk):
                chunks = w_tiles[(s, j)]
                step = DK // len(chunks)
                return chunks[dk // step][:, (dk % step) * SW : (dk % step + 1) * SW]

            interleave0 = bt is not None and len(slot_tiles[0]) > 1
            load_x(slot_tiles[0][0])
            if interleave0:
                # first sweep interleaves the B tile's and first full
                # tile's dk-halves, so x0 rides between the two W chunks
                load_w(0, 0, 2, upto=1)
                load_x(slot_tiles[0][1])
                load_w(0, 0, 2, fromi=1)
                for t in slot_tiles[0][2:]:
                    load_x(t)
            else:
                load_w(0, 0, 2)
                for t in slot_tiles[0][1:]:
                    load_x(t)
            load_w(0, 1, 2)
            load_w(0, 2)
            ident_t = None
            if bt is not None:
                ident_t = wmp.tile([P, P], bf, tag="ident")
                nc.sync.dma_start(ident_t[:], identd[:])
            load_w(0, 3)
            load_w(1, 0)
            for t in slot_tiles[1]:
                load_x(t)
            load_w(1, 1)
            load_w(1, 2)
            load_w(1, 3)

            # --- PE p-state warmup on a memset tile: ramps the tensor
            # engine to full clock while the first DMAs land.  (The bias
            # add happens on the host after the gather, not on-device.)
            dum = wmp.tile([P, P], bf, tag="dum")
            nc.vector.memset(dum[:], 1.0)
            psd = pp.tile([P, SW], f32, tag="ps")
            for _ in range(N_WARM):
                nc.tensor.matmul(
                    psd[:, :P], dum[:], dum[:], start=True, stop=True
                )

            # --- compute: 256-column sweeps, f-major per slot.  Evictions
            # pair two sweeps into one 512-wide output tile; stores ride
            # the otherwise-idle ACT engine except the narrow tail store.
            ot_map = {}

            def group(s, t, j, c0, cw, store_to=None, last=None):
                r = rows_of[t]
                ps = pp.tile([P, SW], f32, tag="ps")
                for dk in range(DK):
                    nc.tensor.matmul(
                        ps[:r, :cw],
                        x_tiles[t][:, dk * r : (dk + 1) * r],
                        w_slice(s, j, dk)[:, c0 : c0 + cw],
                        start=(dk == 0),
                        stop=(dk == DK - 1),
                    )
                if j % 2 == 0 and c0 == 0:
                    otl = op.tile([P, FBLK], bf, tag="o")
                    ot_map[t] = otl
                ot = ot_map[t]
                oc = (j % 2) * SW + c0
                fc = j * SW + c0  # global f column
                nc.vector.tensor_copy(ot[:r, oc : oc + cw], ps[:r, :cw])
                if store_to is not None:
                    # store [a, b) of the 512-wide output tile; the two
                    # tail stores split across SP and DVE so their SEQ+
                    # HWDGE paths don't serialize (loads long done)
                    a, b_ = store_to
                    base = (j - j % 2) * SW
                    eng = {None: nc.scalar, "sp": nc.sync, "act": nc.scalar}[last]
                    eng.dma_start(
                        out[t * P : t * P + r, base + a : base + b_],
                        ot[:r, a:b_],
                    )

            sbB = None
            if bt is not None:
                sbB = wmp.tile([P, 2 * NSW * r0], bf, tag="sbB")

            def group_b(j):
                """Ragged slot-0 tile, transposed orientation: W chunk is
                stationary, the tile's r0 rows are moving (cost ~r0 instead
                of the sweep width), landing [f, row] blocks in PSUM."""
                for fc in range(SW // P):
                    g = j * (SW // P) + fc
                    ps = pp.tile([P, SW], f32, tag="ps")
                    for dk in range(DK):
                        nc.tensor.matmul(
                            ps[:, :r0],
                            w_slice(0, j, dk)[:, fc * P : (fc + 1) * P],
                            x_tiles[bt][:, dk * r0 : (dk + 1) * r0],
                            start=(dk == 0),
                            stop=(dk == DK - 1),
                        )
                    nc.vector.tensor_copy(sbB[:, g * r0 : (g + 1) * r0], ps[:, :r0])

            def finish_b():
                """Re-transpose the 8 [128f, r0] blocks back to row-major
                via the PE and store the tile's rows."""
                otb = None
                for pair in range(D // SW):
                    psT = pp.tile([P, SW], bf, tag="ps")
                    for q in range(2):
                        g = pair * 2 + q
                        nc.tensor.transpose(
                            psT[:r0, q * P : (q + 1) * P],
                            sbB[:, g * r0 : (g + 1) * r0],
                            ident_t[:],
                        )
                    if pair % 2 == 0:
                        otb = op.tile([P, FBLK], bf, tag="o")
                    nc.vector.tensor_copy(
                        otb[:r0, (pair % 2) * SW : (pair % 2 + 1) * SW],
                        psT[:r0, :],
                    )
                    if pair % 2 == 1:
                        base = (pair - 1) * SW
                        nc.scalar.dma_start(
                            out[bt * P : bt * P + r0, base : base + FBLK],
                            otb[:r0, :],
                        )

            def sweep0_interleaved():
                """Sweep 0 of slot 0 with the B tile's and the first full
                tile's dk-halves interleaved: each half starts as soon as
                its 4-dk W chunk lands instead of waiting for all of W00."""
                t0 = slot_tiles[0][1]
                r_t0 = rows_of[t0]
                psb = []
                for _ in range(SW // P):
                    pg = pp.tile([P, SW], f32, tag="ps")
                    psb.append(pg)
                ps0 = pp.tile([P, SW], f32, tag="ps")
                half = DK // 2
                for lo, hi in ((0, half), (half, DK)):
                    for fc in range(SW // P):
                        for dk in range(lo, hi):
                            nc.tensor.matmul(
                                psb[fc][:, :r0],
                                w_slice(0, 0, dk)[:, fc * P : (fc + 1) * P],
                                x_tiles[bt][:, dk * r0 : (dk + 1) * r0],
                                start=(dk == 0),
                                stop=(dk == DK - 1),
                            )
                    for dk in range(lo, hi):
                        nc.tensor.matmul(
                            ps0[:, :SW],
                            x_tiles[t0][:, dk * r_t0 : (dk + 1) * r_t0],
                            w_slice(0, 0, dk),
                            start=(dk == 0),
                            stop=(dk == DK - 1),
                        )
                for fc in range(SW // P):
                    nc.vector.tensor_copy(
                        sbB[:, fc * r0 : (fc + 1) * r0], psb[fc][:, :r0]
                    )
                otl = op.tile([P, FBLK], bf, tag="o")
                ot_map[t0] = otl
                nc.vector.tensor_copy(otl[:r_t0, :SW], ps0[:r_t0, :SW])

            for s in range(SLOTS):
                for j in range(NSW):
                    order = list(slot_tiles[s])
                    if s == 0 and interleave0:
                        if j == 0:
                            sweep0_interleaved()
                            order = slot_tiles[0][2:]
                        else:
                            # B tile second from sweep 1 on, so its dk-
                            # inner groups never wait on a fresh W block
                            order = (
                                [slot_tiles[0][1], bt] + slot_tiles[0][2:]
                            )
                    for t in order:
                        if s == 0 and t == bt:
                            group_b(j)
                            if j == NSW - 1 and len(order) == order.index(bt) + 1:
                                finish_b()
                            continue
                        if (
                            s == SLOTS - 1
                            and j == NSW - 2
                            and t == slot_tiles[s][-1]
                        ):
                            # final tile: store each piece as soon as its
                            # eviction lands so the tail chain holds only
                            # the last narrow 128-column store
                            group(s, t, j, 0, SW, store_to=(0, SW))
                        elif (
                            s == SLOTS - 1
                            and j == NSW - 1
                            and t == slot_tiles[s][-1]
                        ):
                            group(s, t, j, 0, SW - P, store_to=(SW, FBLK - P), last="sp")
                            group(
                                s, t, j, SW - P, P,
                                store_to=(FBLK - P, FBLK), last="sp",
                            )
                        elif j % 2 == 1:
                            group(s, t, j, 0, SW, store_to=(0, FBLK))
                        else:
                            group(s, t, j, 0, SW)
                        if (
                            s == 0
                            and j == NSW - 1
                            and bt is not None
                            and bt in order
                            and order.index(t) == order.index(bt) + 1
                        ):
                            # transpose pass one tile after the last B
                            # group, so its sbB eviction is long done
                            finish_b()

    _nc_cache[key] = nc
    return nc


def _route(cond_i):
    """Expert->slot assignment and per-slot row counts from the routing."""
    counts = np.bincount(cond_i, minlength=C)
    order = np.argsort(-counts, kind="stable")
    slot_experts = (order[:NCORES], order[NCORES:])
    M0 = max(1, int(counts[slot_experts[0]].max()))
    M1 = max(1, int(counts[slot_experts[1]].max()))
    return slot_experts, M0, M1


def build_for_cond(cond):
    """Build (without running) the Bass module for the given routing."""
    cond_i = np.asarray(cond).astype(np.int64)
    _, M0, M1 = _route(cond_i)
    return _build(M0, M1)


def kernel(x, cond, W, b):
    import ml_dtypes

    from concourse.bass_utils import run_bass_kernel_spmd

    global LAST_RESULT, LAST_NC

    bf = ml_dtypes.bfloat16
    x = np.ascontiguousarray(np.asarray(x, dtype=np.float32))
    cond_i = np.asarray(cond).astype(np.int64)
    W = np.asarray(W, dtype=np.float32)
    b = np.asarray(b, dtype=np.float32)

    slot_experts, M0, M1 = _route(cond_i)
    T0 = -(-M0 // P)
    T1 = -(-M1 // P)
    T = T0 + T1

    nc = _build(M0, M1)
    LAST_NC = nc

    idx_by_e = [np.nonzero(cond_i == e)[0] for e in range(C)]
    # program-wide rows held by each tile (ragged tails hold fewer)
    rows_prog = [min(P, M0 - t * P) for t in range(T0)] + [
        min(P, M1 - j * P) for j in range(T1)
    ]
    in_maps = []
    placements = []
    for k in range(NCORES):
        xtk = np.zeros((T, P, DK * P), bf)
        wtk = np.empty((SLOTS, NSW, DK, P, SW), bf)
        for s, base, tbase, Ts in ((0, 0, 0, T0), (1, T0 * P, T0, T1)):
            e = int(slot_experts[s][k])
            idx = idx_by_e[e]
            # wt[s, j, dk, p, f] = W[e][j*256+f, dk*128+p]
            wtk[s] = (
                W[e]
                .T.reshape(DK, P, NSW, SW)
                .transpose(2, 0, 1, 3)
                .astype(bf)
            )
            placements.append((k, base, e))
            for j in range(Ts):
                t = tbase + j
                rp = rows_prog[t]
                rows = idx[j * P : j * P + rp]
                if not len(rows):
                    continue
                # compact layout: xt[t, p, dk*rp + m] = x[rows[m], dk*128+p]
                blk = np.zeros((P, DK, rp), np.float32)
                blk[:, :, : len(rows)] = (
                    x[rows].reshape(len(rows), DK, P).transpose(2, 1, 0)
                )
                xtk[t, :, : DK * rp] = blk.reshape(P, DK * rp).astype(bf)
        in_maps.append(
            {
                "xt": xtk,
                "wt": np.ascontiguousarray(wtk),
                "ident": np.eye(P, dtype=bf),
            }
        )

    res = run_bass_kernel_spmd(nc, in_maps, list(range(NCORES)), trace=TRACE)
    LAST_RESULT = res

    out_full = np.empty((B, D), np.float32)
    for k, base, e in placements:
        idx = idx_by_e[e]
        out_full[idx] = res.results[k]["out"][base : base + len(idx)].astype(
            np.float32
        )
    out_full += b.sum(axis=0)
    return out_full


if __name__ == "__main__":
    rng = np.random.default_rng(0)
    x = rng.standard_normal((B, D), dtype=np.float32)
    cond = rng.integers(0, C, size=B).astype(np.int64)
    W = (rng.standard_normal((C, D, D), dtype=np.float32) / np.sqrt(D)).astype(
        np.float32
    )
    b = (rng.standard_normal((C, D), dtype=np.float32) * 0.02).astype(np.float32)
    got = kernel(x, cond, W, b)
    want = np.empty((B, D), np.float32)
    for e in range(C):
        idx = np.nonzero(cond == e)[0]
        want[idx] = x[idx] @ W[e].T
    want += b.sum(0)
    denom = np.abs(want).max()
    print("max abs err:", np.abs(got - want).max(), "denom:", denom)
    print("rel err:", np.abs(got - want).max() / denom)
